# revision 1
# baseline (speedup 1.0000x reference)
# GCN aggregation (10 layers of normalized scatter-add SpMV) on 8 NeuronCores.
#
# Formulation: with u = D^{-1/2} h, each layer is u' = D^{-1} (A^T u) where
# A^T includes self-loops, and c5_l = ||h_l||^2 = sum(deg * u_l^2).
#
# Sharding: destination cols are dealt round-robin (per in-degree class) to the
# 8 cores. Each core fills a class-padded, col-grouped gather array via
# [128,1]-column indirect DMAs (one offset per partition), reduces each
# class region with a strided tensor_reduce, AllGathers the v chunks, and
# applies the diagonal update. One NEFF computes one layer; the host loop
# invokes it 10 times with u round-tripping through DRAM.
import numpy as np

N = 100000
E = 6400000
NDEV = 8
P = 128
L_LAYERS = 10
CLASSES = list(range(2, 132, 2))

_cache = {}


def legalize_waits(nc):
    # walrus here rejects >1 sem wait per instruction: split extras onto
    # single-wait NoOp carriers inserted before, on the same engine.
    import bass_rust
    import concourse.mybir as mybir
    n = 0
    for blk in nc.m.functions[0].blocks:
        insts = blk.instructions
        i = 0
        while i < len(insts):
            inst = insts[i]
            si = inst.sync_info
            if si is not None and len(si.on_wait) > 1:
                waits = list(si.on_wait)
                si.on_wait = [waits[-1]]
                pre = []
                for w in waits[:-1]:
                    n += 1
                    nop = mybir.InstNoOp(name=f"I-waitfix-{n}", ins=[], outs=[],
                                         text_hint="waitfix")
                    nop.engine = inst.engine
                    nop.sync_info = bass_rust.SyncInfo(on_wait=[w], on_update=[])
                    pre.append(nop)
                insts[i:i] = pre
                i += len(pre)
            i += 1
    return n


def _host_prep(h, edge_index):
    row = np.asarray(edge_index[0], dtype=np.int64)
    col = np.asarray(edge_index[1], dtype=np.int64)
    deg_nl = np.bincount(col, minlength=N).astype(np.int64)   # without self loop
    deg = deg_nl + 1                                          # with self loop

    # sort cols by degree desc; runs of 1024 (128 cols x 8 devices); per-run
    # slot length L_r = max degree in run (tight because sorted)
    order = np.argsort(-deg_nl, kind="stable")
    RUN = P * NDEV
    n_runs = (N + RUN - 1) // RUN
    W = n_runs
    TBL0 = NDEV * P * W
    ZSLOT = TBL0

    L_runs = np.zeros(n_runs, dtype=np.int64)
    for r in range(n_runs):
        seg = order[r * RUN:(r + 1) * RUN]
        L_runs[r] = max(1, int(deg_nl[seg].max()))
    F_max = int(L_runs.sum())
    off_runs = np.zeros(n_runs + 1, dtype=np.int64)
    np.cumsum(L_runs, out=off_runs[1:])

    # new node id: col at run r, position i: device i%8, partition i//8
    newid = np.full(N, -1, dtype=np.int64)
    dev_of = np.full(N, -1, dtype=np.int64)
    part_of = np.full(N, -1, dtype=np.int64)
    run_of = np.full(N, -1, dtype=np.int64)
    for r in range(n_runs):
        seg = order[r * RUN:(r + 1) * RUN]
        i = np.arange(len(seg))
        d = i % NDEV
        p = i // NDEV
        newid[seg] = d * P * W + p * W + r
        dev_of[seg] = d
        part_of[seg] = p
        run_of[seg] = r
    assert (newid >= 0).all()

    # in-edge lists: edges sorted by col
    eorder = np.argsort(col, kind="stable")
    row_s = row[eorder]
    col_s = col[eorder]
    starts = np.zeros(N + 1, dtype=np.int64)
    np.cumsum(np.bincount(col_s, minlength=N), out=starts[1:])
    row_new = newid[row_s]

    # per-device IDX tables [P, F]
    idx_tabs = [np.full((P, F_max), ZSLOT, dtype=np.int32) for _ in range(NDEV)]
    for c in range(N):
        s, e = starts[c], starts[c + 1]
        if e == s:
            continue
        d, p, r = dev_of[c], part_of[c], run_of[c]
        f0 = off_runs[r]
        idx_tabs[d][p, f0:f0 + (e - s)] = row_new[s:e].astype(np.int32)

    deg_flat = np.zeros(TBL0, dtype=np.float32)
    deg_flat[newid] = deg.astype(np.float32)
    dinv2_flat = np.zeros(TBL0, dtype=np.float32)
    dinv2_flat[newid] = (1.0 / deg).astype(np.float32)
    u0_flat = np.zeros(TBL0, dtype=np.float32)
    u0_flat[newid] = (np.asarray(h).ravel() / np.sqrt(deg)).astype(np.float32)

    def to_sb(flat):
        a = flat.reshape(NDEV, P, W)
        return np.transpose(a, (1, 0, 2)).reshape(P, NDEV * W).copy()

    meta = dict(W=W, F=F_max, TBL0=TBL0,
                L_runs=L_runs, off_runs=off_runs, newid=newid)
    arrays = dict(idx_tabs=idx_tabs,
                  deg_sb=to_sb(deg_flat),
                  dinv2_sb=to_sb(dinv2_flat),
                  u0_flat=u0_flat)
    return meta, arrays


def _build_nc(meta):
    import concourse.bass as bass
    import concourse.mybir as mybir
    from concourse.tile import TileContext

    W, F, TBL0 = meta["W"], meta["F"], meta["TBL0"]
    TBL = TBL0 + P  # zero pad region
    WU = NDEV * W

    nc = bass.Bass(num_devices=NDEV)
    utab = nc.dram_tensor("utab", [TBL, 1], mybir.dt.float32, kind="ExternalInput")
    idxt = nc.dram_tensor("idxt", [P, F], mybir.dt.int32, kind="ExternalInput")
    deg_in = nc.dram_tensor("deg", [P, WU], mybir.dt.float32, kind="ExternalInput")
    dinv2_in = nc.dram_tensor("dinv2", [P, WU], mybir.dt.float32, kind="ExternalInput")
    utab_next = nc.dram_tensor("utab_next", [TBL, 1], mybir.dt.float32,
                               kind="ExternalOutput")
    c5_out = nc.dram_tensor("c5", [1, 1], mybir.dt.float32, kind="ExternalOutput")

    vchunk = nc.dram_tensor("vchunk", [P * W], mybir.dt.float32, kind="Internal")
    vgath = nc.dram_tensor("vgath", [NDEV * P * W], mybir.dt.float32,
                           kind="Internal", addr_space="Shared")

    with TileContext(nc) as tc:
        with tc.tile_pool(name="p", bufs=1) as pool, \
             tc.tile_pool(name="ps", bufs=1, space="PSUM") as psum:
            # split gather array + offsets into segments (separate tiles
            # relax Tile's same-tile dependency chaining between idmas)
            L_runs_ = [int(x) for x in meta["L_runs"]]
            off_ = [int(x) for x in meta["off_runs"]]
            NSEG = 8
            tgt = F / NSEG
            seg_run = [0]
            for r in range(len(L_runs_) + 1):
                if len(seg_run) < NSEG and r < len(L_runs_) and off_[r] >= tgt * len(seg_run):
                    seg_run.append(r)
            seg_run.append(len(L_runs_))
            seg_f = [off_[r] for r in seg_run]
            idx_sbs, Bs = [], []
            for s in range(len(seg_run) - 1):
                w = seg_f[s + 1] - seg_f[s]
                it = pool.tile([P, max(w, 1)], mybir.dt.int32, tag=f"idx{s}")
                bt = pool.tile([P, max(w, 1)], mybir.dt.float32, tag=f"B{s}")
                idx_sbs.append(it)
                Bs.append(bt)

            def seg_of(f):
                for s in range(len(seg_f) - 1):
                    if seg_f[s] <= f < seg_f[s + 1]:
                        return s, f - seg_f[s]
                raise ValueError(f)
            v = pool.tile([P, W], mybir.dt.float32, tag="v")
            vfull = pool.tile([P, WU], mybir.dt.float32, tag="vfull")
            dinv2 = pool.tile([P, WU], mybir.dt.float32, tag="dinv2")
            degt = pool.tile([P, WU], mybir.dt.float32, tag="degt")
            un = pool.tile([P, WU], mybir.dt.float32, tag="un")
            sq = pool.tile([P, WU], mybir.dt.float32, tag="sq")
            red = pool.tile([P, 1], mybir.dt.float32, tag="red")
            ones = pool.tile([P, 1], mybir.dt.float32, tag="ones")
            ztail = pool.tile([P, 1], mybir.dt.float32, tag="ztail")
            c5sb = pool.tile([1, 1], mybir.dt.float32, tag="c5sb")

            usb = pool.tile([P, WU], mybir.dt.float32, tag="usb")
            for s in range(len(seg_run) - 1):
                if seg_f[s + 1] > seg_f[s]:
                    nc.sync.dma_start(idx_sbs[s][:], idxt[:, seg_f[s]:seg_f[s + 1]])
            nc.sync.dma_start(
                usb[:].rearrange("p (d w) -> p d w", d=NDEV),
                utab[:TBL0, 0].rearrange("(d p w) -> p d w", d=NDEV, p=P))
            nc.sync.dma_start(dinv2[:], dinv2_in[:])
            nc.sync.dma_start(degt[:], deg_in[:])
            nc.vector.memset(ones[:], 1.0)
            nc.vector.memset(ztail[:], 0.0)

            # 1. B-fill via indirect DMA columns
            with nc.named_scope("bfill"):
                for f in range(F):
                    s, fl = seg_of(f)
                    nc.gpsimd.indirect_dma_start(
                        out=Bs[s][:, fl:fl + 1],
                        out_offset=None,
                        in_=utab[:],
                        in_offset=bass.IndirectOffsetOnAxis(
                            ap=idx_sbs[s][:, fl:fl + 1], axis=0),
                    )

            # 2. per-run reduces (runs with equal L merged into one op)
            with nc.named_scope("reduce"):
                r = 0
                while r < len(L_runs_):
                    Lc = L_runs_[r]
                    r2 = r
                    # stop a merge group at segment boundaries too
                    sseg = seg_of(off_[r])[0]
                    while (r2 < len(L_runs_) and L_runs_[r2] == Lc
                           and seg_of(off_[r2])[0] == sseg):
                        r2 += 1
                    m = r2 - r
                    s, fl = seg_of(off_[r])
                    src_ap = Bs[s][:, fl:fl + m * Lc]
                    src3 = src_ap.rearrange("p (m l) -> p m l", m=m, l=Lc)
                    nc.vector.tensor_reduce(
                        v[:, r:r + m], src3, op=mybir.AluOpType.add,
                        axis=mybir.AxisListType.X)
                    r = r2

            # 3. allgather v
            with nc.named_scope("gather"):
                nc.sync.dma_start(
                    vchunk[:].rearrange("(p w) -> p w", p=P), v[:])
                nc.gpsimd.collective_compute(
                    "AllGather", mybir.AluOpType.bypass,
                    replica_groups=[list(range(NDEV))],
                    ins=[vchunk[:]], outs=[vgath[:]],
                )
                nc.sync.dma_start(
                    vfull[:].rearrange("p (d w) -> p d w", d=NDEV),
                    vgath[:].rearrange("(d p w) -> p d w", d=NDEV, p=P))

            # 4. u' = vfull * dinv2 ; c5 = sum(deg*u'^2)
            with nc.named_scope("update"):
                nc.vector.tensor_tensor(un[:], vfull[:], usb[:],
                                        mybir.AluOpType.add)
                nc.vector.tensor_tensor(un[:], un[:], dinv2[:],
                                        mybir.AluOpType.mult)
                nc.vector.tensor_tensor(sq[:], un[:], un[:],
                                        mybir.AluOpType.mult)
                nc.vector.tensor_tensor(sq[:], sq[:], degt[:],
                                        mybir.AluOpType.mult)
                nc.vector.tensor_reduce(red[:], sq[:],
                                        op=mybir.AluOpType.add,
                                        axis=mybir.AxisListType.X)
                ps = psum.tile([1, 1], mybir.dt.float32)
                nc.tensor.matmul(ps[:], red[:], ones[:], start=True, stop=True)
                nc.vector.tensor_copy(c5sb[:], ps[:])
                nc.sync.dma_start(c5_out[:], c5sb[:])

            # 5. write u' to utab_next (+ zero tail)
            with nc.named_scope("writeback"):
                nc.sync.dma_start(
                    utab_next[:TBL0, 0].rearrange("(d p w) -> p d w",
                                                  d=NDEV, p=P),
                    un[:].rearrange("p (d w) -> p d w", d=NDEV))
                nc.sync.dma_start(
                    utab_next[TBL0:, 0].rearrange("(p one) -> p one", p=P),
                    ztail[:])
    legalize_waits(nc)
    return nc


def kernel(h, edge_index):
    from concourse import bass_utils

    h = np.asarray(h, dtype=np.float32)
    edge_index = np.asarray(edge_index)
    key = "k"
    meta, arrays = _host_prep(h, edge_index)
    if key not in _cache:
        _cache[key] = _build_nc(meta)
    nc = _cache[key]

    TBL = meta["TBL0"] + P
    utab = np.zeros((TBL, 1), dtype=np.float32)
    utab[:meta["TBL0"], 0] = arrays["u0_flat"]
    base = dict(deg=arrays["deg_sb"], dinv2=arrays["dinv2_sb"])
    c5 = []
    for _ in range(L_LAYERS):
        in_maps = [dict(base, utab=utab,
                        idxt=arrays["idx_tabs"][d]) for d in range(NDEV)]
        res = bass_utils.run_bass_kernel_spmd(
            nc, in_maps, core_ids=list(range(NDEV)), trace=False)
        utab = res.results[0]["utab_next"]
        c5.append(float(res.results[0]["c5"][0, 0]))
    return np.asarray(c5, dtype=np.float32)



# revision 3
# speedup vs baseline: 2.8014x; 2.8014x over previous
# GCN aggregation (10 layers of normalized scatter-add SpMV) on 8 NeuronCores.
#
# Formulation: with u = D^{-1/2} h, each layer is u' = D^{-1} (A^T u) where
# A^T includes self-loops, and c5_l = ||h_l||^2 = sum(deg * u_l^2).
#
# Sharding: destination cols are dealt round-robin (per in-degree class) to the
# 8 cores. Each core fills a class-padded, col-grouped gather array B [128, F]
# via 128 per-partition-row indirect DMAs (each row-gather consumes its
# indices COLUMN-MAJOR from a [128, F/128] block of the idx table — measured
# hardware behavior of the vector-indirect DMA with a [1, w, 1] dest AP),
# reduces each class region with strided tensor_reduces, AllGathers the v
# chunks, and applies the diagonal update. All 10 layers run in ONE NEFF;
# u round-trips through per-core DRAM ping-pong buffers between layers.
import numpy as np

N = 100000
E = 6400000
NDEV = 8
P = 128
L_LAYERS = 10

_cache = {}


def legalize_waits(nc):
    # walrus here rejects >1 sem wait per instruction: split extras onto
    # single-wait NoOp carriers inserted before, on the same engine.
    import bass_rust
    import concourse.mybir as mybir
    n = 0
    for blk in nc.m.functions[0].blocks:
        insts = blk.instructions
        i = 0
        while i < len(insts):
            inst = insts[i]
            si = inst.sync_info
            if si is not None and len(si.on_wait) > 1:
                waits = list(si.on_wait)
                si.on_wait = [waits[-1]]
                pre = []
                for w in waits[:-1]:
                    n += 1
                    nop = mybir.InstNoOp(name=f"I-waitfix-{n}", ins=[], outs=[],
                                         text_hint="waitfix")
                    nop.engine = inst.engine
                    nop.sync_info = bass_rust.SyncInfo(on_wait=[w], on_update=[])
                    pre.append(nop)
                insts[i:i] = pre
                i += len(pre)
            i += 1
    return n


def _host_prep(h, edge_index):
    row = np.asarray(edge_index[0], dtype=np.int64)
    col = np.asarray(edge_index[1], dtype=np.int64)
    deg_nl = np.bincount(col, minlength=N).astype(np.int64)   # without self loop
    deg = deg_nl + 1                                          # with self loop

    # sort cols by degree desc; runs of 1024 (128 cols x 8 devices); per-run
    # slot length L_r = max degree in run (tight because sorted)
    order = np.argsort(-deg_nl, kind="stable")
    RUN = P * NDEV
    n_runs = (N + RUN - 1) // RUN
    W = n_runs
    TBL0 = NDEV * P * W
    ZSLOT = TBL0

    L_runs = np.zeros(n_runs, dtype=np.int64)
    for r in range(n_runs):
        seg = order[r * RUN:(r + 1) * RUN]
        L_runs[r] = max(1, int(deg_nl[seg].max()))
    F = int(L_runs.sum())
    FP = ((F + P - 1) // P) * P          # pad to multiple of 128
    CB = FP // P                         # idx columns consumed per row-gather
    off_runs = np.zeros(n_runs + 1, dtype=np.int64)
    np.cumsum(L_runs, out=off_runs[1:])

    # new node id: col at run r, position i: device i%8, partition i//8
    newid = np.full(N, -1, dtype=np.int64)
    dev_of = np.full(N, -1, dtype=np.int64)
    part_of = np.full(N, -1, dtype=np.int64)
    run_of = np.full(N, -1, dtype=np.int64)
    for r in range(n_runs):
        seg = order[r * RUN:(r + 1) * RUN]
        i = np.arange(len(seg))
        d = i % NDEV
        p = i // NDEV
        newid[seg] = d * P * W + p * W + r
        dev_of[seg] = d
        part_of[seg] = p
        run_of[seg] = r
    assert (newid >= 0).all()

    # in-edge lists: edges sorted by col
    eorder = np.argsort(col, kind="stable")
    row_s = row[eorder]
    col_s = col[eorder]
    starts = np.zeros(N + 1, dtype=np.int64)
    np.cumsum(np.bincount(col_s, minlength=N), out=starts[1:])
    row_new = newid[row_s]

    # per-device IDX tables [P, FP] (slot (p, j) reads utab[IDX[p, j]])
    idx_tabs = [np.full((P, FP), ZSLOT, dtype=np.int32) for _ in range(NDEV)]
    for c in range(N):
        s, e = starts[c], starts[c + 1]
        if e == s:
            continue
        d, p, r = dev_of[c], part_of[c], run_of[c]
        f0 = off_runs[r]
        idx_tabs[d][p, f0:f0 + (e - s)] = row_new[s:e].astype(np.int32)

    # hw idx layout: row-gather p consumes cols [p*CB, (p+1)*CB) column-major:
    # consumption t -> idx_hw[t % 128, p*CB + t//128] == IDX[p, t]
    a = np.arange(FP) % P
    b = np.arange(FP) // P
    idx_hws = []
    for d in range(NDEV):
        hw = np.empty((P, FP), dtype=np.int32)
        for p in range(P):
            hw[a, p * CB + b] = idx_tabs[d][p, :]
        idx_hws.append(hw)

    deg_flat = np.zeros(TBL0, dtype=np.float32)
    deg_flat[newid] = deg.astype(np.float32)
    dinv2_flat = np.zeros(TBL0, dtype=np.float32)
    dinv2_flat[newid] = (1.0 / deg).astype(np.float32)
    u0_flat = np.zeros(TBL0, dtype=np.float32)
    u0_flat[newid] = (np.asarray(h).ravel() / np.sqrt(deg)).astype(np.float32)

    def to_sb(flat):
        arr = flat.reshape(NDEV, P, W)
        return np.transpose(arr, (1, 0, 2)).reshape(P, NDEV * W).copy()

    meta = dict(W=W, F=F, FP=FP, CB=CB, TBL0=TBL0,
                L_runs=L_runs, off_runs=off_runs, newid=newid)
    arrays = dict(idx_hws=idx_hws,
                  deg_sb=to_sb(deg_flat),
                  dinv2_sb=to_sb(dinv2_flat),
                  u0_flat=u0_flat)
    return meta, arrays


def _build_nc(meta):
    import concourse.bass as bass
    import concourse.mybir as mybir
    from concourse.tile import TileContext

    W, F, FP, CB, TBL0 = meta["W"], meta["F"], meta["FP"], meta["CB"], meta["TBL0"]
    TBL = TBL0 + P  # zero pad region
    WU = NDEV * W

    nc = bass.Bass(num_devices=NDEV)
    utab_in = nc.dram_tensor("utab", [TBL, 1], mybir.dt.float32,
                             kind="ExternalInput")
    idxt = nc.dram_tensor("idxt", [P, FP], mybir.dt.int32, kind="ExternalInput")
    deg_in = nc.dram_tensor("deg", [P, WU], mybir.dt.float32, kind="ExternalInput")
    dinv2_in = nc.dram_tensor("dinv2", [P, WU], mybir.dt.float32,
                              kind="ExternalInput")
    c5_out = nc.dram_tensor("c5", [1, L_LAYERS], mybir.dt.float32,
                            kind="ExternalOutput")

    upp = [nc.dram_tensor(f"upp{i}", [TBL, 1], mybir.dt.float32, kind="Internal")
           for i in range(2)]
    vchunk = nc.dram_tensor("vchunk", [P * W], mybir.dt.float32, kind="Internal")
    vgath = nc.dram_tensor("vgath", [NDEV * P * W], mybir.dt.float32,
                           kind="Internal", addr_space="Shared")

    L_runs_ = [int(x) for x in meta["L_runs"]]
    off_ = [int(x) for x in meta["off_runs"]]

    with TileContext(nc) as tc:
        with tc.tile_pool(name="p", bufs=1) as pool, \
             tc.tile_pool(name="ps", bufs=1, space="PSUM") as psum:
            idx_sb = pool.tile([P, FP], mybir.dt.int32, tag="idx")
            B = pool.tile([P, FP], mybir.dt.float32, tag="B")
            v = pool.tile([P, W], mybir.dt.float32, tag="v")
            vfull = pool.tile([P, WU], mybir.dt.float32, tag="vfull")
            dinv2 = pool.tile([P, WU], mybir.dt.float32, tag="dinv2")
            degt = pool.tile([P, WU], mybir.dt.float32, tag="degt")
            usb = pool.tile([P, WU], mybir.dt.float32, tag="usb")
            un = pool.tile([P, WU], mybir.dt.float32, tag="un")
            sq = pool.tile([P, WU], mybir.dt.float32, tag="sq")
            red = pool.tile([P, 1], mybir.dt.float32, tag="red")
            ones = pool.tile([P, 1], mybir.dt.float32, tag="ones")
            ztail = pool.tile([P, 1], mybir.dt.float32, tag="ztail")
            c5sb = pool.tile([1, L_LAYERS], mybir.dt.float32, tag="c5sb")

            nc.sync.dma_start(idx_sb[:], idxt[:])
            nc.sync.dma_start(
                usb[:].rearrange("p (d w) -> p d w", d=NDEV),
                utab_in[:TBL0, 0].rearrange("(d p w) -> p d w", d=NDEV, p=P))
            nc.sync.dma_start(dinv2[:], dinv2_in[:])
            nc.sync.dma_start(degt[:], deg_in[:])
            nc.vector.memset(ones[:], 1.0)
            nc.vector.memset(ztail[:], 0.0)
            # zero tails of the ping-pong buffers once
            for i in range(2):
                nc.sync.dma_start(
                    upp[i][TBL0:, 0].rearrange("(p one) -> p one", p=P), ztail[:])

            for layer in range(L_LAYERS):
                src = utab_in if layer == 0 else upp[layer % 2]
                dst = upp[(layer + 1) % 2]

                # 1. B-fill: one per-element indirect row-gather per partition
                with nc.named_scope(f"bfill{layer}"):
                    for p in range(P):
                        nc.gpsimd.indirect_dma_start(
                            out=B[p:p + 1, :].rearrange("p (w one) -> p w one",
                                                        one=1),
                            out_offset=None,
                            in_=src[:],
                            in_offset=bass.IndirectOffsetOnAxis(
                                ap=idx_sb[:, p * CB:(p + 1) * CB], axis=0),
                        )

                # 2. per-run reduces (runs with equal L merged into one op)
                with nc.named_scope(f"reduce{layer}"):
                    r = 0
                    while r < len(L_runs_):
                        Lc = L_runs_[r]
                        r2 = r
                        while r2 < len(L_runs_) and L_runs_[r2] == Lc:
                            r2 += 1
                        m = r2 - r
                        src3 = B[:, off_[r]:off_[r] + m * Lc].rearrange(
                            "p (m l) -> p m l", m=m, l=Lc)
                        nc.vector.tensor_reduce(
                            v[:, r:r + m], src3, op=mybir.AluOpType.add,
                            axis=mybir.AxisListType.X)
                        r = r2

                # 3. allgather v
                with nc.named_scope(f"gather{layer}"):
                    nc.sync.dma_start(
                        vchunk[:].rearrange("(p w) -> p w", p=P), v[:])
                    nc.gpsimd.collective_compute(
                        "AllGather", mybir.AluOpType.bypass,
                        replica_groups=[list(range(NDEV))],
                        ins=[vchunk[:]], outs=[vgath[:]],
                    )
                    nc.sync.dma_start(
                        vfull[:].rearrange("p (d w) -> p d w", d=NDEV),
                        vgath[:].rearrange("(d p w) -> p d w", d=NDEV, p=P))

                # 4. u' = (vfull + u) * dinv2 ; c5_l = sum(deg*u'^2)
                with nc.named_scope(f"update{layer}"):
                    nc.vector.tensor_tensor(un[:], vfull[:], usb[:],
                                            mybir.AluOpType.add)
                    nc.vector.tensor_tensor(un[:], un[:], dinv2[:],
                                            mybir.AluOpType.mult)
                    nc.vector.tensor_tensor(sq[:], un[:], un[:],
                                            mybir.AluOpType.mult)
                    nc.vector.tensor_tensor(sq[:], sq[:], degt[:],
                                            mybir.AluOpType.mult)
                    nc.vector.tensor_reduce(red[:], sq[:],
                                            op=mybir.AluOpType.add,
                                            axis=mybir.AxisListType.X)
                    ps = psum.tile([1, 1], mybir.dt.float32)
                    nc.tensor.matmul(ps[:], red[:], ones[:], start=True,
                                     stop=True)
                    nc.vector.tensor_copy(c5sb[:, layer:layer + 1], ps[:])
                    nc.vector.tensor_copy(usb[:], un[:])

                # 5. write u' to the next gather table (skip on last layer)
                if layer + 1 < L_LAYERS:
                    with nc.named_scope(f"writeback{layer}"):
                        nc.sync.dma_start(
                            dst[:TBL0, 0].rearrange("(d p w) -> p d w",
                                                    d=NDEV, p=P),
                            un[:].rearrange("p (d w) -> p d w", d=NDEV))

            nc.sync.dma_start(c5_out[:], c5sb[:])
    legalize_waits(nc)
    return nc


def kernel(h, edge_index):
    from concourse import bass_utils

    h = np.asarray(h, dtype=np.float32)
    edge_index = np.asarray(edge_index)
    meta, arrays = _host_prep(h, edge_index)
    key = "k"
    if key not in _cache:
        _cache[key] = _build_nc(meta)
    nc = _cache[key]

    TBL = meta["TBL0"] + P
    utab = np.zeros((TBL, 1), dtype=np.float32)
    utab[:meta["TBL0"], 0] = arrays["u0_flat"]
    in_maps = [dict(deg=arrays["deg_sb"], dinv2=arrays["dinv2_sb"],
                    utab=utab, idxt=arrays["idx_hws"][d]) for d in range(NDEV)]
    res = bass_utils.run_bass_kernel_spmd(
        nc, in_maps, core_ids=list(range(NDEV)), trace=False)
    return np.asarray(res.results[0]["c5"][0], dtype=np.float32)
